# revision 11
# baseline (speedup 1.0000x reference)
"""Trainium2 Bass kernel for a 3D non-local attention block.

Math (per batch b):
  xf = x.reshape(C, N)                         C=64, N=32768 (=32^3)
  theta = w_theta @ xf                         [8, N]
  phi   = maxpool2(w_phi @ xf)                 [8, M], M=4096
  g     = maxpool2(w_g   @ xf)                 [32, M]
  beta  = softmax_over_m(theta^T phi)          [N, M]
  o     = g @ beta^T                           [32, N]
  out   = gamma * (w_o @ o) + xf               [C, N]

Sharding: 8 cores, core k -> batch k//4, query slice k%4 (8192 queries).
Every core re-computes the (cheap) pooled phi/g from the full batch and
runs flash-style attention over its own query slice; no collectives.

On-device layout: scores are produced transposed [m(part), n(free)] so
exp runs on ScalarE straight out of PSUM and the second matmul consumes
exp(S) with no transposes; the softmax denominator falls out of the
same matmul as a 33rd row (ones column appended to g^T).
"""

import os
import sys

sys.path.insert(0, "/opt/trn_rl_repo")

import numpy as np

C = 64            # channels
N = 32768         # voxels (32^3)
NS = N // 4       # query slice per core (8192)
M = N // 8        # pooled keys (4096)
F = 512           # free-dim tile (PSUM bank)
NT = NS // F      # 16 n-tiles per core
MC = M // 128     # 32 m-chunks of 128
GROUPS = [(s, min(s + 3, MC)) for s in range(0, MC, 3)]  # 3-chunk exp groups


def _build_program(mm_dt_name="float32r"):
    import concourse.bass as bass  # noqa: F401
    import concourse.tile as tile
    from concourse import bacc, mybir
    from concourse.masks import make_identity

    f32 = mybir.dt.float32
    mmdt = getattr(mybir.dt, mm_dt_name)

    def mm(ap):
        return ap

    nc = bacc.Bacc()

    x_full = nc.declare_dram_parameter("x_full", [C, N], f32, isOutput=False)
    x_slice = nc.declare_dram_parameter("x_slice", [C, NS], f32, isOutput=False)
    w_pg = nc.declare_dram_parameter("w_pg", [C, 40], f32, isOutput=False)
    w_th = nc.declare_dram_parameter("w_th", [C, 8], f32, isOutput=False)
    w_oT = nc.declare_dram_parameter("w_oT", [32, C], f32, isOutput=False)
    gamma = nc.declare_dram_parameter("gamma", [1, 1], f32, isOutput=False)
    out_d = nc.declare_dram_parameter("out", [C, NS], f32, isOutput=True)

    Exp = mybir.ActivationFunctionType.Exp
    Max = mybir.AluOpType.max

    with tile.TileContext(nc) as tc:
        with (
            tc.tile_pool(name="consts", bufs=1) as consts,
            tc.tile_pool(name="big", bufs=1) as bigpool,
            tc.tile_pool(name="pool2", bufs=1) as pool2,
            tc.tile_pool(name="theta", bufs=1) as thpool,
            tc.tile_pool(name="pg", bufs=1) as pgpool,
            tc.tile_pool(name="xin", bufs=2) as xpool,
            tc.tile_pool(name="small", bufs=2) as smallpool,
            tc.tile_pool(name="outp", bufs=3) as outpool,
        ):
            w_pg_sb = consts.tile([C, 40], mmdt)
            nc.gpsimd.dma_start(out=w_pg_sb, in_=w_pg[:])
            w_th_sb = consts.tile([C, 8], mmdt)
            nc.gpsimd.dma_start(out=w_th_sb, in_=w_th[:])
            w_oT_sb = consts.tile([32, C], mmdt)
            nc.gpsimd.dma_start(out=w_oT_sb, in_=w_oT[:])
            gamma_sb = consts.tile([1, 1], f32)
            nc.sync.dma_start(out=gamma_sb, in_=gamma[:])
            ident = consts.tile([32, 32], f32)
            make_identity(nc, ident)
            ones32 = consts.tile([128, 32], f32)
            nc.vector.memset(ones32, 1.0)

            # pooled g (w_pg rows 0:32) and phi (rows 32:40); separate tiles so PE
            # operands sit at base partition 0 and slices stay 32-aligned
            phi_sb = pgpool.tile([8, M], mmdt)
            g_sb = pgpool.tile([32, M], f32)

            # ---- Phase A: fused phi/g projection + 2x2x2 maxpool,
            # processed in two 16384-column halves (d in [16h, 16h+16)).
            with tc.tile_pool(name="psA", bufs=4, space="PSUM") as psA:
                for h in range(2):
                    pre = bigpool.tile([40, 16384], f32, tag="big")
                    for cch in range(8):  # 2048-col x chunks
                        base = h * 16384 + cch * 2048
                        xc = xpool.tile([C, 2048], mmdt, tag="x")
                        nc.gpsimd.dma_start(out=xc, in_=x_full[:, base : base + 2048])
                        for k in range(4):
                            ps = psA.tile([40, F], f32, tag="psA")
                            nc.tensor.matmul(
                                ps,
                                mm(w_pg_sb),
                                mm(xc[:, k * F : (k + 1) * F]),
                                start=True,
                                stop=True,
                            )
                            nc.vector.tensor_copy(
                                pre[:, cch * 2048 + k * F : cch * 2048 + (k + 1) * F],
                                ps,
                            )
                    # pool w-pairs: [40, 16, 32, 16, 2] -> [40, 8192]
                    s1 = thpool.tile([40, 8192], f32, tag="th32")
                    v = pre.rearrange("c (m two) -> c m two", two=2)
                    nc.vector.tensor_tensor(s1, v[:, :, 0], v[:, :, 1], Max)
                    # pool h-pairs: [40, 16, 16, 2, 16] -> [40, 4096]
                    s2 = pool2.tile([40, 4096], f32, tag="mid16")
                    v = s1.rearrange(
                        "c (d hh two w) -> c d hh two w", d=16, hh=16, two=2, w=16
                    )
                    nc.vector.tensor_tensor(
                        s2, v[:, :, :, 0, :], v[:, :, :, 1, :], Max
                    )
                    # pool d-pairs: [40, 8, 2, 256] -> [40, 2048]
                    v = s2.rearrange("c (d two r) -> c d two r", d=8, two=2, r=256)
                    nc.vector.tensor_tensor(
                        g_sb[:, h * 2048 : (h + 1) * 2048],
                        v[0:32, :, 0, :],
                        v[0:32, :, 1, :],
                        Max,
                    )
                    nc.vector.tensor_tensor(
                        phi_sb[:, h * 2048 : (h + 1) * 2048],
                        v[32:40, :, 0, :],
                        v[32:40, :, 1, :],
                        Max,
                    )

                # ---- theta projection over this core's query slice
                theta_sb = thpool.tile([8, NS], mmdt, tag="th32")
                for cch in range(4):
                    xc = xpool.tile([C, 2048], mmdt, tag="x")
                    nc.gpsimd.dma_start(
                        out=xc, in_=x_slice[:, cch * 2048 : (cch + 1) * 2048]
                    )
                    for k in range(4):
                        ps = psA.tile([8, F], f32, tag="psTh")
                        nc.tensor.matmul(
                            ps,
                            mm(w_th_sb),
                            mm(xc[:, k * F : (k + 1) * F]),
                            start=True,
                            stop=True,
                        )
                        nc.vector.tensor_copy(
                            theta_sb[:, cch * 2048 + k * F : cch * 2048 + (k + 1) * F],
                            ps,
                        )

            # ---- Phase B: G' = [g^T | 1] in [128, MC, 33] chunk-major
            gt = pool2.tile([128, MC, 33], mmdt, tag="mid16")
            with tc.tile_pool(name="psB", bufs=4, space="PSUM") as psB:
                for j in range(MC):
                    tps = psB.tile([128, 32], f32, tag="psB")
                    nc.tensor.transpose(
                        tps, g_sb[:, j * 128 : (j + 1) * 128], ident
                    )
                    nc.vector.tensor_copy(gt[:, j, 0:32], tps)
            nc.vector.tensor_copy(gt[:, :, 32], ones32)

            # ---- Phase C: flash attention over 16 n-tiles
            with (
                tc.tile_pool(name="psS", bufs=2, space="PSUM") as psS,
                tc.tile_pool(name="psO", bufs=1, space="PSUM") as psO,
                tc.tile_pool(name="psP", bufs=1, space="PSUM") as psP,
            ):
                for t in range(NT):
                    n0 = t * F
                    expS = bigpool.tile([128, MC, F], mmdt, tag="big")
                    o_ps = psO.tile([33, F], f32)
                    pending = None  # (group index range awaiting o-matmuls)
                    for gi, (mc0, mc1) in enumerate(GROUPS):
                        cnt = mc1 - mc0
                        sps = psS.tile([128, 3 * F], f32, tag="psS")
                        for i, mc in enumerate(range(mc0, mc1)):
                            nc.tensor.matmul(
                                sps[:, i * F : (i + 1) * F],
                                mm(phi_sb[:, mc * 128 : (mc + 1) * 128]),
                                mm(theta_sb[:, n0 : n0 + F]),
                                start=True,
                                stop=True,
                            )
                        nc.scalar.activation(
                            out=expS[:, mc0:mc1, :], in_=sps[:, 0 : cnt * F], func=Exp
                        )
                        if pending is not None:
                            for mc in range(*pending):
                                nc.tensor.matmul(
                                    o_ps,
                                    mm(gt[:, mc, :]),
                                    mm(expS[:, mc, :]),
                                    start=(mc == 0),
                                    stop=False,
                                )
                        pending = (mc0, mc1)
                    for mc in range(*pending):
                        nc.tensor.matmul(
                            o_ps,
                            mm(gt[:, mc, :]),
                            mm(expS[:, mc, :]),
                            start=False,
                            stop=(mc == MC - 1),
                        )

                    # normalize + gamma, project, residual, store
                    r1 = smallpool.tile([1, F], f32, tag="r1")
                    nc.vector.reciprocal(r1, o_ps[32:33, :])
                    nc.vector.tensor_scalar_mul(r1, r1, gamma_sb[0:1, 0:1])
                    rb = smallpool.tile([32, F], f32, tag="rb")
                    nc.gpsimd.partition_broadcast(rb, r1)
                    o_sb = smallpool.tile([32, F], mmdt, tag="osb")
                    nc.vector.tensor_mul(o_sb, o_ps[0:32, :], rb)
                    pps = psP.tile([C, F], f32)
                    nc.tensor.matmul(pps, mm(w_oT_sb), mm(o_sb), start=True, stop=True)
                    xres = xpool.tile([C, F], f32, tag="xres")
                    nc.sync.dma_start(out=xres, in_=x_slice[:, n0 : n0 + F])
                    ot = outpool.tile([C, F], f32)
                    nc.vector.tensor_add(ot, pps, xres)
                    nc.sync.dma_start(out=out_d[:, n0 : n0 + F], in_=ot)

    nc.finalize()
    return nc


def _maybe_trace_setup():
    """Optional NTFF profiling (test harness only, via NLATTN_TRACE=1)."""
    if not os.environ.get("NLATTN_TRACE"):
        return False
    import types

    try:
        from antenv.axon_hooks import get_axon_ntff_profile_hook  # noqa: F401
    except ImportError:
        import antenv

        mod = types.ModuleType("antenv.axon_hooks")
        mod._hook = None

        def set_axon_ntff_profile_hook(h):
            mod._hook = h

        def get_axon_ntff_profile_hook():
            return mod._hook

        mod.set_axon_ntff_profile_hook = set_axon_ntff_profile_hook
        mod.get_axon_ntff_profile_hook = get_axon_ntff_profile_hook
        sys.modules["antenv.axon_hooks"] = mod
        antenv.axon_hooks = mod
        from trn_agent_boot.trn_boot import _ntff_profile_via_ctypes

        mod._hook = _ntff_profile_via_ctypes("/opt/axon/libaxon_pjrt.so")
    import concourse.bass_utils as bu

    bu.upload_artifacts = lambda tmpdir: "local://" + str(tmpdir)
    return True


_LAST_RESULT = {}


def kernel(x, w_theta, w_phi, w_g, w_o, gamma):
    from concourse.bass_utils import run_bass_kernel_spmd

    trace = _maybe_trace_setup()

    B = x.shape[0]
    xf = np.ascontiguousarray(x.reshape(B, C, N), dtype=np.float32)
    w_pg_h = np.ascontiguousarray(
        np.concatenate([w_g, w_phi], axis=0).T, dtype=np.float32
    )
    w_th_h = np.ascontiguousarray(np.asarray(w_theta).T, dtype=np.float32)
    w_oT_h = np.ascontiguousarray(np.asarray(w_o).T, dtype=np.float32)
    gamma_h = np.asarray(gamma, dtype=np.float32).reshape(1, 1)

    nc = _build_program(os.environ.get("NLATTN_MM_DT", "float32r"))

    in_maps = []
    for core in range(8):
        b, s = core // 4, core % 4
        in_maps.append(
            {
                "x_full": xf[b],
                "x_slice": np.ascontiguousarray(xf[b][:, s * NS : (s + 1) * NS]),
                "w_pg": w_pg_h,
                "w_th": w_th_h,
                "w_oT": w_oT_h,
                "gamma": gamma_h,
            }
        )

    res = run_bass_kernel_spmd(
        nc, in_maps, core_ids=list(range(8)), trace=trace
    )
    _LAST_RESULT["exec_time_ns"] = res.exec_time_ns
    _LAST_RESULT["trace"] = res.instructions_and_trace

    out = np.empty((B, C, N), dtype=np.float32)
    for core in range(8):
        b, s = core // 4, core % 4
        out[b][:, s * NS : (s + 1) * NS] = res.results[core]["out"]
    D = H = W = 32
    return out.reshape(B, C, D, H, W)


# revision 19
# speedup vs baseline: 1.0978x; 1.0978x over previous
"""Trainium2 Bass kernel for a 3D non-local attention block.

Math (per batch b):
  xf = x.reshape(C, N)                         C=64, N=32768 (=32^3)
  theta = w_theta @ xf                         [8, N]
  phi   = maxpool2(w_phi @ xf)                 [8, M], M=4096
  g     = maxpool2(w_g   @ xf)                 [32, M]
  beta  = softmax_over_m(theta^T phi)          [N, M]
  o     = g @ beta^T                           [32, N]
  out   = gamma * (w_o @ o) + xf               [C, N]

Sharding: 8 cores, core k -> batch k//4, query slice k%4 (8192 queries).
Every core re-computes the (cheap) pooled phi/g from the full batch and
runs flash-style attention over its own query slice; no collectives.

On-device layout: scores are produced transposed [m(part), n(free)] so
exp runs on ScalarE straight out of PSUM and the second matmul consumes
exp(S) with no transposes; the softmax denominator falls out of the
same matmul as a 33rd row (ones column appended to g^T).
"""

import os
import sys

sys.path.insert(0, "/opt/trn_rl_repo")

import numpy as np

C = 64            # channels
N = 32768         # voxels (32^3)
NS = N // 4       # query slice per core (8192)
M = N // 8        # pooled keys (4096)
F = 512           # free-dim tile (PSUM bank)
NT = NS // F      # 16 n-tiles per core
MC = M // 128     # 32 m-chunks of 128
GROUPS = [(s, min(s + 3, MC)) for s in range(0, MC, 3)]  # 3-chunk exp groups


def _build_program(mm_dt_name="float32r"):
    import concourse.bass as bass  # noqa: F401
    import concourse.tile as tile
    from concourse import bacc, mybir
    from concourse.masks import make_identity

    f32 = mybir.dt.float32
    bf16 = mybir.dt.bfloat16
    mmdt = getattr(mybir.dt, mm_dt_name)

    def mm(ap):
        return ap

    nc = bacc.Bacc()

    x_full = nc.declare_dram_parameter("x_full", [C, N], f32, isOutput=False)
    x_slice = nc.declare_dram_parameter("x_slice", [C, NS], f32, isOutput=False)
    w_pg = nc.declare_dram_parameter("w_pg", [C, 40], f32, isOutput=False)
    w_th = nc.declare_dram_parameter("w_th", [C, 8], f32, isOutput=False)
    w_oT = nc.declare_dram_parameter("w_oT", [32, C], f32, isOutput=False)
    gamma = nc.declare_dram_parameter("gamma", [1, 1], f32, isOutput=False)
    out_d = nc.declare_dram_parameter("out", [C, NS], f32, isOutput=True)

    Exp = mybir.ActivationFunctionType.Exp
    Max = mybir.AluOpType.max

    with tile.TileContext(nc) as tc:
        with (
            tc.tile_pool(name="consts", bufs=1) as consts,
            tc.tile_pool(name="big", bufs=1) as bigpool,
            tc.tile_pool(name="pool2", bufs=1) as pool2,
            tc.tile_pool(name="theta", bufs=1) as thpool,
            tc.tile_pool(name="pg", bufs=1) as pgpool,
            tc.tile_pool(name="xin", bufs=2) as xpool,
            tc.tile_pool(name="small", bufs=2) as smallpool,
            tc.tile_pool(name="outp", bufs=2) as outpool,
        ):
            w_pg_sb = consts.tile([C, 40], mmdt)
            nc.gpsimd.dma_start(out=w_pg_sb, in_=w_pg[:])
            w_th_sb = consts.tile([C, 8], mmdt)
            nc.gpsimd.dma_start(out=w_th_sb, in_=w_th[:])
            w_oT_sb = consts.tile([32, C], mmdt)
            nc.gpsimd.dma_start(out=w_oT_sb, in_=w_oT[:])
            gamma_sb = consts.tile([1, 1], f32)
            nc.sync.dma_start(out=gamma_sb, in_=gamma[:])
            ident = consts.tile([32, 32], f32)
            make_identity(nc, ident)
            ones32 = consts.tile([128, 32], f32)
            nc.vector.memset(ones32, 1.0)
            zeros_sb = consts.tile([128, F], f32)
            nc.vector.memset(zeros_sb, 0.0)

            def zero_fill(dst_tile, parts, free):
                for z0 in range(0, free, F):
                    zc = min(F, free - z0)
                    nc.vector.tensor_copy(
                        dst_tile[0:parts, z0 : z0 + zc], zeros_sb[0:parts, 0:zc]
                    )

            # pooled g (w_pg rows 0:32) and phi (rows 32:40); separate tiles so PE
            # operands sit at base partition 0 and slices stay 32-aligned
            phi_sb = pgpool.tile([96, M], mmdt)   # rows {0,32,64}+0:8 = copies, rest 0
            g_sb = pgpool.tile([32, M], f32)
            zero_fill(phi_sb, 96, M)

            # ---- Phase A: fused phi/g projection + 2x2x2 maxpool,
            # processed in two 16384-column halves (d in [16h, 16h+16)).
            with tc.tile_pool(name="psA", bufs=4, space="PSUM") as psA:
                for h in range(2):
                    pre = bigpool.tile([40, 16384], f32, tag="big")
                    for cch in range(8):  # 2048-col x chunks
                        base = h * 16384 + cch * 2048
                        xc = xpool.tile([C, 2048], mmdt, tag="x")
                        nc.gpsimd.dma_start(out=xc, in_=x_full[:, base : base + 2048])
                        for k in range(4):
                            ps = psA.tile([40, F], f32, tag="psA")
                            nc.tensor.matmul(
                                ps,
                                mm(w_pg_sb),
                                mm(xc[:, k * F : (k + 1) * F]),
                                start=True,
                                stop=True,
                            )
                            nc.vector.tensor_copy(
                                pre[:, cch * 2048 + k * F : cch * 2048 + (k + 1) * F],
                                ps,
                            )
                    # pool w-pairs: [40, 16, 32, 16, 2] -> [40, 8192]
                    s1 = thpool.tile([40, 8192], f32, tag="th96")
                    v = pre.rearrange("c (m two) -> c m two", two=2)
                    nc.vector.tensor_tensor(s1, v[:, :, 0], v[:, :, 1], Max)
                    # pool h-pairs: [40, 16, 16, 2, 16] -> [40, 4096]
                    s2 = pool2.tile([40, 4096], f32, tag="mid16")
                    v = s1.rearrange(
                        "c (d hh two w) -> c d hh two w", d=16, hh=16, two=2, w=16
                    )
                    nc.vector.tensor_tensor(
                        s2, v[:, :, :, 0, :], v[:, :, :, 1, :], Max
                    )
                    # pool d-pairs: [40, 8, 2, 256] -> [40, 2048]
                    v = s2.rearrange("c (d two r) -> c d two r", d=8, two=2, r=256)
                    nc.vector.tensor_tensor(
                        g_sb[:, h * 2048 : (h + 1) * 2048],
                        v[0:32, :, 0, :],
                        v[0:32, :, 1, :],
                        Max,
                    )
                    nc.vector.tensor_tensor(
                        phi_sb[0:8, h * 2048 : (h + 1) * 2048],
                        v[32:40, :, 0, :],
                        v[32:40, :, 1, :],
                        Max,
                    )

                # ---- theta projection over this core's query slice
                theta_sb = thpool.tile([96, NS], mmdt, tag="th96")
                zero_fill(theta_sb, 96, NS)
                for cch in range(4):
                    xc = xpool.tile([C, 2048], mmdt, tag="x")
                    nc.gpsimd.dma_start(
                        out=xc, in_=x_slice[:, cch * 2048 : (cch + 1) * 2048]
                    )
                    for k in range(4):
                        ps = psA.tile([8, F], f32, tag="psTh")
                        nc.tensor.matmul(
                            ps,
                            mm(w_th_sb),
                            mm(xc[:, k * F : (k + 1) * F]),
                            start=True,
                            stop=True,
                        )
                        nc.vector.tensor_copy(
                            theta_sb[0:8, cch * 2048 + k * F : cch * 2048 + (k + 1) * F],
                            ps,
                        )

            # replicate theta/phi to partition offsets 32/64 for row-tiled S
            for off in (32, 64):
                nc.sync.dma_start(
                    out=theta_sb[off : off + 8, :], in_=theta_sb[0:8, :]
                )
                nc.sync.dma_start(out=phi_sb[off : off + 8, :], in_=phi_sb[0:8, :])

            # ---- Phase B: G' = [g^T | 1] in [128, MC, 33] chunk-major
            gt = pool2.tile([128, MC, 64], bf16, tag="mid16")
            zero_fill(gt.rearrange("p a b -> p (a b)"), 128, MC * 64)
            with tc.tile_pool(name="psB", bufs=4, space="PSUM") as psB:
                for j in range(MC):
                    tps = psB.tile([128, 32], f32, tag="psB")
                    nc.tensor.transpose(
                        tps, g_sb[:, j * 128 : (j + 1) * 128], ident
                    )
                    nc.vector.tensor_copy(gt[:, j, 0:32], tps)
            nc.vector.tensor_copy(gt[:, :, 32], ones32)

            # ---- Phase C: flash attention over 16 n-tiles
            with (
                tc.tile_pool(name="psS", bufs=2, space="PSUM") as psS,
                tc.tile_pool(name="psO", bufs=1, space="PSUM") as psO_p,
                tc.tile_pool(name="psP", bufs=1, space="PSUM") as psP,
            ):
                for t in range(NT):
                    n0 = t * F
                    expS = bigpool.tile([128, MC, F], bf16, tag="big")
                    psO = psO_p.tile([128, F], f32)   # parity-0 o accumulator
                    po2 = psP.tile([128, F], f32)     # [64:97] parity-1, [0:64] proj
                    pending = None
                    for gi, (mc0, mc1) in enumerate(GROUPS):
                        cnt = mc1 - mc0
                        sps = psS.tile([128, 3 * F], f32, tag="psS")
                        for i, mc in enumerate(range(mc0, mc1)):
                            nc.tensor.matmul(
                                sps[:, i * F : (i + 1) * F],
                                mm(phi_sb[32 * i : 32 * i + 8, mc * 128 : (mc + 1) * 128]),
                                mm(theta_sb[32 * i : 32 * i + 8, n0 : n0 + F]),
                                start=True,
                                stop=True,
                                tile_position=(32 * i, 0),
                            )
                        nc.scalar.activation(
                            out=expS[:, mc0:mc1, :], in_=sps[:, 0 : cnt * F], func=Exp
                        )
                        if pending is not None:
                            for mc in range(*pending):
                                par = mc % 2
                                nc.tensor.matmul(
                                    psO[0:64, :] if par == 0 else po2[64:128, :],
                                    mm(gt[:, mc, :]),
                                    mm(expS[:, mc, :]),
                                    start=(mc < 2),
                                    stop=(mc >= MC - 2),
                                    tile_position=(0, 0) if par == 0 else (0, 64),
                                )
                        pending = (mc0, mc1)
                    for mc in range(*pending):
                        par = mc % 2
                        nc.tensor.matmul(
                            psO[0:64, :] if par == 0 else po2[64:128, :],
                            mm(gt[:, mc, :]),
                            mm(expS[:, mc, :]),
                            start=(mc < 2),
                            stop=(mc >= MC - 2),
                            tile_position=(0, 0) if par == 0 else (0, 64),
                        )

                    # merge parities, normalize + gamma, project, residual
                    o_b = smallpool.tile([33, F], f32, tag="ob")
                    nc.vector.tensor_copy(o_b, po2[64:97, :])
                    den = smallpool.tile([1, F], f32, tag="den")
                    nc.vector.tensor_tensor(
                        den, psO[32:33, :], o_b[32:33, :], mybir.AluOpType.add
                    )
                    nc.vector.reciprocal(den, den)
                    nc.vector.tensor_scalar_mul(den, den, gamma_sb[0:1, 0:1])
                    rb = smallpool.tile([32, F], f32, tag="rb")
                    nc.gpsimd.partition_broadcast(rb, den)
                    o_m = smallpool.tile([32, F], f32, tag="om")
                    nc.vector.tensor_tensor(
                        o_m, psO[0:32, :], o_b[0:32, :], mybir.AluOpType.add
                    )
                    o_sb = smallpool.tile([32, F], mmdt, tag="osb")
                    nc.vector.tensor_mul(o_sb, o_m, rb)
                    nc.tensor.matmul(
                        po2[0:64, :], mm(w_oT_sb), mm(o_sb), start=True, stop=True
                    )
                    xres = xpool.tile([C, F], f32, tag="x")
                    nc.sync.dma_start(out=xres, in_=x_slice[:, n0 : n0 + F])
                    ot = outpool.tile([C, F], f32)
                    nc.vector.tensor_add(ot, po2[0:64, :], xres)
                    nc.sync.dma_start(out=out_d[:, n0 : n0 + F], in_=ot)

    nc.finalize()
    return nc


def _maybe_trace_setup():
    """Optional NTFF profiling (test harness only, via NLATTN_TRACE=1)."""
    if not os.environ.get("NLATTN_TRACE"):
        return False
    import types

    try:
        from antenv.axon_hooks import get_axon_ntff_profile_hook  # noqa: F401
    except ImportError:
        import antenv

        mod = types.ModuleType("antenv.axon_hooks")
        mod._hook = None

        def set_axon_ntff_profile_hook(h):
            mod._hook = h

        def get_axon_ntff_profile_hook():
            return mod._hook

        mod.set_axon_ntff_profile_hook = set_axon_ntff_profile_hook
        mod.get_axon_ntff_profile_hook = get_axon_ntff_profile_hook
        sys.modules["antenv.axon_hooks"] = mod
        antenv.axon_hooks = mod
        from trn_agent_boot.trn_boot import _ntff_profile_via_ctypes

        mod._hook = _ntff_profile_via_ctypes("/opt/axon/libaxon_pjrt.so")
    import concourse.bass_utils as bu

    bu.upload_artifacts = lambda tmpdir: "local://" + str(tmpdir)
    return True


_LAST_RESULT = {}


def kernel(x, w_theta, w_phi, w_g, w_o, gamma):
    from concourse.bass_utils import run_bass_kernel_spmd

    trace = _maybe_trace_setup()

    B = x.shape[0]
    xf = np.ascontiguousarray(x.reshape(B, C, N), dtype=np.float32)
    w_pg_h = np.ascontiguousarray(
        np.concatenate([w_g, w_phi], axis=0).T, dtype=np.float32
    )
    w_th_h = np.ascontiguousarray(np.asarray(w_theta).T, dtype=np.float32)
    w_oT_h = np.ascontiguousarray(np.asarray(w_o).T, dtype=np.float32)
    gamma_h = np.asarray(gamma, dtype=np.float32).reshape(1, 1)

    nc = _build_program(os.environ.get("NLATTN_MM_DT", "float32r"))

    in_maps = []
    for core in range(8):
        b, s = core // 4, core % 4
        in_maps.append(
            {
                "x_full": xf[b],
                "x_slice": np.ascontiguousarray(xf[b][:, s * NS : (s + 1) * NS]),
                "w_pg": w_pg_h,
                "w_th": w_th_h,
                "w_oT": w_oT_h,
                "gamma": gamma_h,
            }
        )

    res = run_bass_kernel_spmd(
        nc, in_maps, core_ids=list(range(8)), trace=trace
    )
    _LAST_RESULT["exec_time_ns"] = res.exec_time_ns
    _LAST_RESULT["trace"] = res.instructions_and_trace

    out = np.empty((B, C, N), dtype=np.float32)
    for core in range(8):
        b, s = core // 4, core % 4
        out[b][:, s * NS : (s + 1) * NS] = res.results[core]["out"]
    D = H = W = 32
    return out.reshape(B, C, D, H, W)
